# revision 9
# baseline (speedup 1.0000x reference)
"""Trainium2 Bass kernel for nn_AttentionHead (B=4, S=2048, D_IN=D_OUT=1024).

Sharding: 8 cores; core c handles batch b=c//2 and half the queries
(balanced for causal load): even cores q in [0,512)+[1536,2048), odd cores
q in [512,1536).  Each core computes full K^T and V projections for its
batch (duplicated within the pair) and causal attention for its queries,
as two uniform 512-query phase slots with K_slot=(8,16) key-tiles; causal
masking and the slot padding are data-driven (host-sent thresholds), so
all 8 cores run one identical SPMD program.

All matmuls are fp32r (full-rate TensorE at free-dim 512, ~2e-4 rel err).
Everything is computed transposed so no on-chip transposes are needed:
  K^T[e,k] = Wk-tiles.T @ Xk^T,  Q^T[e,q] = Wq-tiles.T @ Xq^T
  (host pre-transposes X into SBUF-ready [dp, do, s] blocks),
  V[k,e] = Xv^T-tiles.T @ Wv   (staged to DRAM, streamed back per e-tile),
  S^T[k,q] = KT-tiles.T @ QT,  exp+causal-mask on S^T,
  den[*,q] = ones.T @ expS (replicated on all partitions),
  O^T[e,q] = V-tiles.T @ expS^T, multiplied by 1/den.
"""
import sys
import types

sys.path.insert(0, "/opt/trn_rl_repo")


def _install_ntff_hook():
    import antenv

    if "antenv.axon_hooks" in sys.modules:
        return
    mod = types.ModuleType("antenv.axon_hooks")
    _h = [None]
    mod.set_axon_ntff_profile_hook = lambda h: _h.__setitem__(0, h)
    mod.get_axon_ntff_profile_hook = lambda: _h[0]
    sys.modules["antenv.axon_hooks"] = mod
    antenv.axon_hooks = mod
    try:
        from trn_agent_boot.trn_boot import _ntff_profile_via_ctypes

        mod.set_axon_ntff_profile_hook(
            _ntff_profile_via_ctypes("/opt/axon/libaxon_pjrt.so"))
    except Exception:
        pass


_install_ntff_hook()

# Walrus codegen on this toolchain supports a single sync-wait per
# instruction; collapse each DGE class onto one semaphore so Tile emits one
# merged wait per consumer (loads ride SWDGE via gpsimd, stores ride HWDGE
# via nc.sync so PE load-waits don't include store traffic).
import concourse.tile_sem_assignment as _tsa

_tsa.NUM_SWDGE_GLOBAL_SEMS = 1
_tsa.NUM_HWDGE_SEMS = 1
import concourse.tile_scheduler as _tsch

_tsch.NUM_SWDGE_GLOBAL_SEMS = 1
_tsch.NUM_HWDGE_SEMS = 1

import numpy as np
import concourse.bass as bass
import concourse.tile as tile
from concourse import mybir
from concourse.bass_utils import run_bass_kernel_spmd

P = 128
B, S, D = 4, 2048, 1024
N = 512                      # matmul moving free dim / queries per slot
NCORES = 8
K_SLOTS = (8, 16)            # k-tiles per phase slot (uniform across cores)
Q0S = {0: (0, 1536), 1: (512, 1024)}   # slot query starts per core parity
SCALE = float(1.0 / np.sqrt(np.float32(2048)))

f32 = mybir.dt.float32
f32r = mybir.dt.float32r
EXP = mybir.ActivationFunctionType.Exp
MULT = mybir.AluOpType.mult


def _split_multi_waits(nc):
    """Walrus allows one sync-wait per instruction; split extras onto
    wait-only NoOps inserted right before the offending instruction."""
    for f in nc.m.functions:
        for bb in f.blocks:
            insts = bb.instructions
            i = 0
            while i < len(insts):
                ins = insts[i]
                si = getattr(ins, "sync_info", None)
                if si and si.on_wait and len(si.on_wait) > 1:
                    waits = list(si.on_wait)
                    for j, w in enumerate(waits[:-1]):
                        nop = mybir.InstNoOp(
                            name=f"{ins.name}-waitsplit-{j}",
                            sync_info=mybir.SyncInfo(on_wait=[w], on_update=[]),
                            bass_nofuse=True,
                            engine=ins.engine, ins=[], outs=[])
                        insts.insert(i + j, nop)
                    i += len(waits) - 1
                    ins.sync_info = mybir.SyncInfo(
                        on_wait=[waits[-1]], on_update=list(si.on_update))
                i += 1


def build():
    nc = bass.Bass()
    # all host-side tensors are pre-arranged into SBUF layout [dp, do, cols]
    wq = nc.dram_tensor("wq", [P, 8, D], f32r, kind="ExternalInput")
    wk = nc.dram_tensor("wk", [P, 8, D], f32r, kind="ExternalInput")
    wv = nc.dram_tensor("wv", [P, 8, D], f32r, kind="ExternalInput")
    xqt = nc.dram_tensor("xqt", [P, 8, 1024], f32r, kind="ExternalInput")
    xkt = nc.dram_tensor("xkt", [P, 8, S], f32r, kind="ExternalInput")
    xvt = nc.dram_tensor("xvt", [P, 8, S], f32r, kind="ExternalInput")
    thr = nc.dram_tensor("thr", [P, 2, 16], f32, kind="ExternalInput")
    iot = nc.dram_tensor("iota", [P, N], f32, kind="ExternalInput")
    one_in = nc.dram_tensor("ones", [P, P], f32r, kind="ExternalInput")
    out = nc.dram_tensor("out", [D, 1024], f32, kind="ExternalOutput")

    with tile.TileContext(nc) as tc:
        from contextlib import ExitStack
        with ExitStack() as ctx:
            kt_pool = ctx.enter_context(tc.tile_pool(name="ktp", bufs=1))
            xh_pool = ctx.enter_context(tc.tile_pool(name="xh", bufs=2))
            sm_pool = ctx.enter_context(tc.tile_pool(name="sm", bufs=1))
            psum = ctx.enter_context(
                tc.tile_pool(name="ps", bufs=8, space="PSUM"))
            dram = ctx.enter_context(
                tc.tile_pool(name="dram", bufs=1, space="DRAM"))

            v_dram = dram.tile([P, 16, D], f32r)    # V: [k_p, k_o, e]

            KT = kt_pool.tile([P, 8, S], f32r)      # K^T: [e_p, e_o, k]

            ones = sm_pool.tile([P, P], f32r)
            nc.gpsimd.dma_start(ones[:], one_in[:])
            iota_sb = sm_pool.tile([P, N], f32)
            nc.gpsimd.dma_start(iota_sb[:], iot[:])
            thr_sb = sm_pool.tile([P, 2, 16], f32)
            nc.gpsimd.dma_start(thr_sb[:], thr[:])

            ET_GROUPS = ((0, 3), (3, 6), (6, 8))

            # ---- Stage A: K^T[e,k] = sum_d Wk-tiles.T @ Xk^T[d,k] ----
            # ---- Stage B: V[k,e]  = sum_d Xv^T-tiles.T @ Wv[d,e]  ----
            with tc.tile_pool(name="wres", bufs=1) as wres, \
                    tc.tile_pool(name="vp", bufs=4) as v_pool:
                wk_sb = wres.tile([P, 8, D], f32r, tag="w")
                for d in range(8):
                    nc.gpsimd.dma_start(wk_sb[:, d, :], wk[:, d, :])
                for half in range(2):
                    xk_h = xh_pool.tile([P, 8, 1024], f32r, tag="xh",
                                        name=f"xk{half}")
                    for d in range(8):
                        nc.gpsimd.dma_start(
                            xk_h[:, d, :],
                            xkt[:, d, half * 1024:(half + 1) * 1024])
                    # d-outer so the first matmul only needs strip d=0
                    for g0, g1 in ET_GROUPS:
                        pss = {}
                        for et in range(g0, g1):
                            for kc in range(2):
                                pss[(et, kc)] = psum.tile(
                                    [P, N], f32, tag="ps",
                                    name=f"psa{half}_{et}_{kc}")
                        for d in range(8):
                            for et in range(g0, g1):
                                lhs = wk_sb[:, d, et * P:(et + 1) * P]
                                for kc in range(2):
                                    nc.tensor.matmul(
                                        pss[(et, kc)][:], lhs,
                                        xk_h[:, d, kc * N:(kc + 1) * N],
                                        start=(d == 0), stop=(d == 7))
                        for et in range(g0, g1):
                            for kc in range(2):
                                col = half * 1024 + kc * N
                                nc.vector.tensor_copy(
                                    KT[:, et, col:col + N], pss[(et, kc)][:])

                wv_sb = wres.tile([P, 8, D], f32r, tag="w")
                for d in range(8):
                    nc.gpsimd.dma_start(wv_sb[:, d, :], wv[:, d, :])
                for half in range(2):
                    xv_h = xh_pool.tile([P, 8, 1024], f32r, tag="xh",
                                        name=f"xv{half}")
                    for d in range(8):
                        nc.gpsimd.dma_start(
                            xv_h[:, d, :],
                            xvt[:, d, half * 1024:(half + 1) * 1024])
                    for ktl in range(8):
                        ktg = half * 8 + ktl
                        ps2 = [psum.tile([P, N], f32, tag="ps",
                                         name=f"psb{half}_{ktl}_{i}")
                               for i in range(2)]
                        for d in range(8):
                            lhs = xv_h[:, d, ktl * P:(ktl + 1) * P]
                            for ec in range(2):
                                nc.tensor.matmul(
                                    ps2[ec][:], lhs,
                                    wv_sb[:, d, ec * N:(ec + 1) * N],
                                    start=(d == 0), stop=(d == 7))
                        for ec in range(2):
                            vt = v_pool.tile([P, N], f32r, tag="vst")
                            nc.vector.tensor_copy(vt[:], ps2[ec][:])
                            nc.sync.dma_start(
                                v_dram[:, ktg, ec * N:(ec + 1) * N], vt[:])

            # ---- Stage D: per phase slot: Q^T, scores, softmax, O^T ----
            qt_pool = ctx.enter_context(tc.tile_pool(name="qtp", bufs=1))
            wq_pool = ctx.enter_context(tc.tile_pool(name="wqp", bufs=6))
            vin_pool = ctx.enter_context(tc.tile_pool(name="vin", bufs=3))
            out_pool = ctx.enter_context(tc.tile_pool(name="op", bufs=2))
            mk_pool = ctx.enter_context(tc.tile_pool(name="mk", bufs=2))
            rd_pool = ctx.enter_context(tc.tile_pool(name="rd", bufs=2))
            for s in range(2):
                K = K_SLOTS[s]
                # Q^T for this slot
                xq_s = xh_pool.tile([P, 8, N], f32r, tag="xh",
                                    name=f"xq{s}")
                nc.gpsimd.dma_start(xq_s[:], xqt[:, :, s * N:(s + 1) * N])
                QT = qt_pool.tile([P, 8, N], f32r, tag="qt", name=f"qt{s}")
                for et in range(8):
                    psq = psum.tile([P, N], f32, tag="ps", name=f"psq{s}_{et}")
                    for d in range(8):
                        wqt = wq_pool.tile([P, P], f32r, tag="wqt",
                                           name=f"wqt{s}_{et}_{d}")
                        nc.gpsimd.dma_start(
                            wqt[:], wq[:, d, et * P:(et + 1) * P])
                        nc.tensor.matmul(psq[:], wqt[:], xq_s[:, d, :],
                                         start=(d == 0), stop=(d == 7))
                    nc.vector.tensor_copy(QT[:, et, :], psq[:])

                # scores^T -> exp -> causal/pad mask
                expS = xh_pool.tile([P, 16, N], f32r, tag="xh",
                                    name=f"expS{s}")
                for kt in range(K):
                    ps = psum.tile([P, N], f32, tag="ps", name=f"pss{s}_{kt}")
                    for ec in range(8):
                        nc.tensor.matmul(
                            ps[:], KT[:, ec, kt * P:(kt + 1) * P],
                            QT[:, ec, :],
                            start=(ec == 0), stop=(ec == 7))
                    nc.scalar.activation(expS[:, kt, :], ps[:], EXP,
                                         scale=SCALE)
                    if not (s == 1 and kt < 8):
                        mk = mk_pool.tile([P, N], f32r)
                        nc.vector.tensor_scalar(
                            out=mk[:], in0=iota_sb[:],
                            scalar1=thr_sb[:, s, kt:kt + 1], scalar2=None,
                            op0=mybir.AluOpType.is_ge)
                        nc.vector.tensor_tensor(
                            out=expS[:, kt, :], in0=expS[:, kt, :],
                            in1=mk[:], op=MULT)

                # denominator, replicated on all partitions
                dps = psum.tile([P, N], f32, tag="ps", name=f"dps{s}")
                for kt in range(K):
                    nc.tensor.matmul(dps[:], ones[:], expS[:, kt, :],
                                     start=(kt == 0), stop=(kt == K - 1))
                rden = rd_pool.tile([P, N], f32)
                nc.vector.reciprocal(rden[:], dps[:])

                # O^T[e,q] with per-et V slabs streamed from DRAM
                for et in range(8):
                    slab = vin_pool.tile([P, 16, P], f32r, tag="vs",
                                         name=f"vs{s}_{et}")
                    nc.gpsimd.dma_start(
                        slab[:, :K, :], v_dram[:, :K, et * P:(et + 1) * P])
                    po = psum.tile([P, N], f32, tag="ps", name=f"po{s}_{et}")
                    for kt in range(K):
                        nc.tensor.matmul(po[:], slab[:, kt, :],
                                         expS[:, kt, :],
                                         start=(kt == 0), stop=(kt == K - 1))
                    ot = out_pool.tile([P, N], f32)
                    nc.vector.tensor_tensor(out=ot[:], in0=po[:],
                                            in1=rden[:], op=MULT)
                    nc.sync.dma_start(
                        out[et * P:(et + 1) * P, s * N:(s + 1) * N], ot[:])

    _split_multi_waits(nc)
    return nc


_NC_CACHE = None


def _get_nc():
    global _NC_CACHE
    if _NC_CACHE is None:
        _NC_CACHE = build()
    return _NC_CACHE


def _sbufize(a):
    """[rows(1024), cols] -> [dp(128), do(8), cols] contiguous."""
    r, c = a.shape
    return np.ascontiguousarray(a.reshape(8, P, c).transpose(1, 0, 2))


def _host_prep(inputs_for_keys, inputs_for_values, inputs_for_queries,
               weight_q, weight_k, weight_v):
    f = lambda a: np.asarray(a, dtype=np.float32)
    ik, iv, iq = f(inputs_for_keys), f(inputs_for_values), f(inputs_for_queries)
    wq = _sbufize(f(weight_q))
    wk = _sbufize(f(weight_k))
    wv = _sbufize(f(weight_v))

    iota = np.broadcast_to(np.arange(N, dtype=np.float32), (P, N)).copy()
    onesm = np.ones((P, P), np.float32)
    in_maps = []
    for c in range(NCORES):
        b, h = c // 2, c % 2
        q0s = Q0S[h]
        xq = np.concatenate([iq[b, q0:q0 + 512] for q0 in q0s], axis=0)
        x = np.arange(P, dtype=np.float32)
        thr = np.empty((P, 2, 16), np.float32)
        for s_, q0 in enumerate(q0s):
            for kt in range(16):
                thr[:, s_, kt] = kt * P + x - q0
        in_maps.append({
            "wq": wq, "wk": wk, "wv": wv,
            "xqt": _sbufize(np.ascontiguousarray(xq.T)),
            "xkt": _sbufize(np.ascontiguousarray(ik[b].T)),
            "xvt": _sbufize(np.ascontiguousarray(iv[b].T)),
            "thr": thr, "iota": iota, "ones": onesm,
        })
    return in_maps


def _assemble(results):
    out = np.empty((B, S, D), np.float32)
    for c in range(NCORES):
        b, h = c // 2, c % 2
        oc = results[c]["out"].T        # [q_local, e]
        for s_, q0 in enumerate(Q0S[h]):
            out[b, q0:q0 + 512] = oc[s_ * 512:(s_ + 1) * 512]
    return out


def kernel(**inputs) -> np.ndarray:
    nc = _get_nc()
    in_maps = _host_prep(**inputs)
    res = run_bass_kernel_spmd(nc, in_maps, list(range(NCORES)))
    return _assemble(res.results)


def kernel_profiled(**inputs):
    """Like kernel() but also returns (output, exec_time_ns, results)."""
    nc = _get_nc()
    in_maps = _host_prep(**inputs)
    res = run_bass_kernel_spmd(nc, in_maps, list(range(NCORES)), trace=True)
    return _assemble(res.results), res.exec_time_ns, res


# revision 10
# speedup vs baseline: 2.1914x; 2.1914x over previous
"""Trainium2 Bass kernel for nn_AttentionHead (B=4, S=2048, D_IN=D_OUT=1024).

Sharding: 8 cores; core c handles batch b=c//2 and half the queries
(balanced for causal load): even cores q in [0,512)+[1536,2048), odd cores
q in [512,1536).  Each core computes full K^T and V projections for its
batch (duplicated within the pair) and causal attention for its queries,
as two uniform 512-query phase slots with K_slot=(8,16) key-tiles; causal
masking and the slot padding are data-driven (host-sent thresholds), so
all 8 cores run one identical SPMD program.

All matmuls are fp32r (full-rate TensorE at free-dim 512, ~2e-4 rel err).
Everything is computed transposed so no on-chip transposes are needed:
  K^T[e,k] = Wk-tiles.T @ Xk^T,  Q^T[e,q] = Wq-tiles.T @ Xq^T
  (host pre-transposes X into SBUF-ready [dp, do, s] blocks),
  V[k,e] = Xv^T-tiles.T @ Wv   (staged to DRAM, streamed back per e-tile),
  S^T[k,q] = KT-tiles.T @ QT,  exp+causal-mask on S^T,
  den[*,q] = ones.T @ expS (replicated on all partitions),
  O^T[e,q] = V-tiles.T @ expS^T, multiplied by 1/den.
"""
import sys
import types

sys.path.insert(0, "/opt/trn_rl_repo")


def _install_ntff_hook():
    import antenv

    if "antenv.axon_hooks" in sys.modules:
        return
    mod = types.ModuleType("antenv.axon_hooks")
    _h = [None]
    mod.set_axon_ntff_profile_hook = lambda h: _h.__setitem__(0, h)
    mod.get_axon_ntff_profile_hook = lambda: _h[0]
    sys.modules["antenv.axon_hooks"] = mod
    antenv.axon_hooks = mod
    try:
        from trn_agent_boot.trn_boot import _ntff_profile_via_ctypes

        mod.set_axon_ntff_profile_hook(
            _ntff_profile_via_ctypes("/opt/axon/libaxon_pjrt.so"))
    except Exception:
        pass


_install_ntff_hook()


import numpy as np
import concourse.bass as bass
import concourse.tile as tile
from concourse import mybir
from concourse.bass_utils import run_bass_kernel_spmd

P = 128
B, S, D = 4, 2048, 1024
N = 512                      # matmul moving free dim / queries per slot
NCORES = 8
K_SLOTS = (8, 16)            # k-tiles per phase slot (uniform across cores)
Q0S = {0: (0, 1536), 1: (512, 1024)}   # slot query starts per core parity
SCALE = float(1.0 / np.sqrt(np.float32(2048)))

f32 = mybir.dt.float32
f32r = mybir.dt.float32r
EXP = mybir.ActivationFunctionType.Exp
MULT = mybir.AluOpType.mult


def _split_multi_waits(nc):
    """Walrus allows one sync-wait per instruction; split extras onto
    wait-only NoOps inserted right before the offending instruction."""
    for f in nc.m.functions:
        for bb in f.blocks:
            insts = bb.instructions
            i = 0
            while i < len(insts):
                ins = insts[i]
                si = getattr(ins, "sync_info", None)
                if si and si.on_wait and len(si.on_wait) > 1:
                    waits = list(si.on_wait)
                    for j, w in enumerate(waits[:-1]):
                        nop = mybir.InstNoOp(
                            name=f"{ins.name}-waitsplit-{j}",
                            sync_info=mybir.SyncInfo(on_wait=[w], on_update=[]),
                            bass_nofuse=True,
                            engine=ins.engine, ins=[], outs=[])
                        insts.insert(i + j, nop)
                    i += len(waits) - 1
                    ins.sync_info = mybir.SyncInfo(
                        on_wait=[waits[-1]], on_update=list(si.on_update))
                i += 1


def build():
    nc = bass.Bass()
    # all host-side tensors are pre-arranged into SBUF layout [dp, do, cols]
    wq = nc.dram_tensor("wq", [P, 8, D], f32r, kind="ExternalInput")
    wk = nc.dram_tensor("wk", [P, 8, D], f32r, kind="ExternalInput")
    wv = nc.dram_tensor("wv", [P, 8, D], f32r, kind="ExternalInput")
    xqt = nc.dram_tensor("xqt", [P, 8, 1024], f32r, kind="ExternalInput")
    xkt = nc.dram_tensor("xkt", [P, 8, S], f32r, kind="ExternalInput")
    xvt = nc.dram_tensor("xvt", [P, 8, S], f32r, kind="ExternalInput")
    thr = nc.dram_tensor("thr", [P, 2, 16], f32, kind="ExternalInput")
    iot = nc.dram_tensor("iota", [P, N], f32, kind="ExternalInput")
    one_in = nc.dram_tensor("ones", [P, P], f32r, kind="ExternalInput")
    out = nc.dram_tensor("out", [D, 1024], f32, kind="ExternalOutput")

    with tile.TileContext(nc) as tc:
        from contextlib import ExitStack
        with ExitStack() as ctx:
            kt_pool = ctx.enter_context(tc.tile_pool(name="ktp", bufs=1))
            xh_pool = ctx.enter_context(tc.tile_pool(name="xh", bufs=2))
            sm_pool = ctx.enter_context(tc.tile_pool(name="sm", bufs=1))
            psum = ctx.enter_context(
                tc.tile_pool(name="ps", bufs=8, space="PSUM"))
            dram = ctx.enter_context(
                tc.tile_pool(name="dram", bufs=1, space="DRAM"))

            v_dram = dram.tile([P, 16, D], f32r)    # V: [k_p, k_o, e]

            KT = kt_pool.tile([P, 8, S], f32r)      # K^T: [e_p, e_o, k]

            ones = sm_pool.tile([P, P], f32r)
            nc.gpsimd.dma_start(ones[:], one_in[:])
            iota_sb = sm_pool.tile([P, N], f32)
            nc.gpsimd.dma_start(iota_sb[:], iot[:])
            thr_sb = sm_pool.tile([P, 2, 16], f32)
            nc.gpsimd.dma_start(thr_sb[:], thr[:])

            ET_GROUPS = ((0, 3), (3, 6), (6, 8))

            # ---- Stage A: K^T[e,k] = sum_d Wk-tiles.T @ Xk^T[d,k] ----
            # ---- Stage B: V[k,e]  = sum_d Xv^T-tiles.T @ Wv[d,e]  ----
            with tc.tile_pool(name="wres", bufs=1) as wres, \
                    tc.tile_pool(name="vp", bufs=4) as v_pool:
                wk_sb = wres.tile([P, 8, D], f32r, tag="w")
                for d in range(8):
                    nc.gpsimd.dma_start(wk_sb[:, d, :], wk[:, d, :])
                for half in range(2):
                    xk_h = xh_pool.tile([P, 8, 1024], f32r, tag="xh",
                                        name=f"xk{half}")
                    for d in range(8):
                        nc.gpsimd.dma_start(
                            xk_h[:, d, :],
                            xkt[:, d, half * 1024:(half + 1) * 1024])
                    # d-outer so the first matmul only needs strip d=0
                    for g0, g1 in ET_GROUPS:
                        pss = {}
                        for et in range(g0, g1):
                            for kc in range(2):
                                pss[(et, kc)] = psum.tile(
                                    [P, N], f32, tag="ps",
                                    name=f"psa{half}_{et}_{kc}")
                        for d in range(8):
                            for et in range(g0, g1):
                                lhs = wk_sb[:, d, et * P:(et + 1) * P]
                                for kc in range(2):
                                    nc.tensor.matmul(
                                        pss[(et, kc)][:], lhs,
                                        xk_h[:, d, kc * N:(kc + 1) * N],
                                        start=(d == 0), stop=(d == 7))
                        for et in range(g0, g1):
                            for kc in range(2):
                                col = half * 1024 + kc * N
                                nc.vector.tensor_copy(
                                    KT[:, et, col:col + N], pss[(et, kc)][:])

                wv_sb = wres.tile([P, 8, D], f32r, tag="w")
                for d in range(8):
                    nc.gpsimd.dma_start(wv_sb[:, d, :], wv[:, d, :])
                for half in range(2):
                    xv_h = xh_pool.tile([P, 8, 1024], f32r, tag="xh",
                                        name=f"xv{half}")
                    for d in range(8):
                        nc.gpsimd.dma_start(
                            xv_h[:, d, :],
                            xvt[:, d, half * 1024:(half + 1) * 1024])
                    for ktl in range(8):
                        ktg = half * 8 + ktl
                        ps2 = [psum.tile([P, N], f32, tag="ps",
                                         name=f"psb{half}_{ktl}_{i}")
                               for i in range(2)]
                        for d in range(8):
                            lhs = xv_h[:, d, ktl * P:(ktl + 1) * P]
                            for ec in range(2):
                                nc.tensor.matmul(
                                    ps2[ec][:], lhs,
                                    wv_sb[:, d, ec * N:(ec + 1) * N],
                                    start=(d == 0), stop=(d == 7))
                        for ec in range(2):
                            vt = v_pool.tile([P, N], f32r, tag="vst")
                            nc.vector.tensor_copy(vt[:], ps2[ec][:])
                            nc.sync.dma_start(
                                v_dram[:, ktg, ec * N:(ec + 1) * N], vt[:])

            # ---- Stage D: per phase slot: Q^T, scores, softmax, O^T ----
            qt_pool = ctx.enter_context(tc.tile_pool(name="qtp", bufs=1))
            wq_pool = ctx.enter_context(tc.tile_pool(name="wqp", bufs=6))
            vin_pool = ctx.enter_context(tc.tile_pool(name="vin", bufs=3))
            out_pool = ctx.enter_context(tc.tile_pool(name="op", bufs=2))
            mk_pool = ctx.enter_context(tc.tile_pool(name="mk", bufs=2))
            rd_pool = ctx.enter_context(tc.tile_pool(name="rd", bufs=2))
            for s in range(2):
                K = K_SLOTS[s]
                # Q^T for this slot
                xq_s = xh_pool.tile([P, 8, N], f32r, tag="xh",
                                    name=f"xq{s}")
                nc.gpsimd.dma_start(xq_s[:], xqt[:, :, s * N:(s + 1) * N])
                QT = qt_pool.tile([P, 8, N], f32r, tag="qt", name=f"qt{s}")
                for et in range(8):
                    psq = psum.tile([P, N], f32, tag="ps", name=f"psq{s}_{et}")
                    for d in range(8):
                        wqt = wq_pool.tile([P, P], f32r, tag="wqt",
                                           name=f"wqt{s}_{et}_{d}")
                        nc.gpsimd.dma_start(
                            wqt[:], wq[:, d, et * P:(et + 1) * P])
                        nc.tensor.matmul(psq[:], wqt[:], xq_s[:, d, :],
                                         start=(d == 0), stop=(d == 7))
                    nc.vector.tensor_copy(QT[:, et, :], psq[:])

                # scores^T -> exp -> causal/pad mask
                expS = xh_pool.tile([P, 16, N], f32r, tag="xh",
                                    name=f"expS{s}")
                for kt in range(K):
                    ps = psum.tile([P, N], f32, tag="ps", name=f"pss{s}_{kt}")
                    for ec in range(8):
                        nc.tensor.matmul(
                            ps[:], KT[:, ec, kt * P:(kt + 1) * P],
                            QT[:, ec, :],
                            start=(ec == 0), stop=(ec == 7))
                    nc.scalar.activation(expS[:, kt, :], ps[:], EXP,
                                         scale=SCALE)
                    if not (s == 1 and kt < 8):
                        mk = mk_pool.tile([P, N], f32r)
                        nc.vector.tensor_scalar(
                            out=mk[:], in0=iota_sb[:],
                            scalar1=thr_sb[:, s, kt:kt + 1], scalar2=None,
                            op0=mybir.AluOpType.is_ge)
                        nc.vector.tensor_tensor(
                            out=expS[:, kt, :], in0=expS[:, kt, :],
                            in1=mk[:], op=MULT)

                # denominator, replicated on all partitions
                dps = psum.tile([P, N], f32, tag="ps", name=f"dps{s}")
                for kt in range(K):
                    nc.tensor.matmul(dps[:], ones[:], expS[:, kt, :],
                                     start=(kt == 0), stop=(kt == K - 1))
                rden = rd_pool.tile([P, N], f32)
                nc.vector.reciprocal(rden[:], dps[:])

                # O^T[e,q] with per-et V slabs streamed from DRAM
                for et in range(8):
                    slab = vin_pool.tile([P, 16, P], f32r, tag="vs",
                                         name=f"vs{s}_{et}")
                    nc.gpsimd.dma_start(
                        slab[:, :K, :], v_dram[:, :K, et * P:(et + 1) * P])
                    po = psum.tile([P, N], f32, tag="ps", name=f"po{s}_{et}")
                    for kt in range(K):
                        nc.tensor.matmul(po[:], slab[:, kt, :],
                                         expS[:, kt, :],
                                         start=(kt == 0), stop=(kt == K - 1))
                    ot = out_pool.tile([P, N], f32)
                    nc.vector.tensor_tensor(out=ot[:], in0=po[:],
                                            in1=rden[:], op=MULT)
                    nc.sync.dma_start(
                        out[et * P:(et + 1) * P, s * N:(s + 1) * N], ot[:])

    _split_multi_waits(nc)
    return nc


_NC_CACHE = None


def _get_nc():
    global _NC_CACHE
    if _NC_CACHE is None:
        _NC_CACHE = build()
    return _NC_CACHE


def _sbufize(a):
    """[rows(1024), cols] -> [dp(128), do(8), cols] contiguous."""
    r, c = a.shape
    return np.ascontiguousarray(a.reshape(8, P, c).transpose(1, 0, 2))


def _host_prep(inputs_for_keys, inputs_for_values, inputs_for_queries,
               weight_q, weight_k, weight_v):
    f = lambda a: np.asarray(a, dtype=np.float32)
    ik, iv, iq = f(inputs_for_keys), f(inputs_for_values), f(inputs_for_queries)
    wq = _sbufize(f(weight_q))
    wk = _sbufize(f(weight_k))
    wv = _sbufize(f(weight_v))

    iota = np.broadcast_to(np.arange(N, dtype=np.float32), (P, N)).copy()
    onesm = np.ones((P, P), np.float32)
    in_maps = []
    for c in range(NCORES):
        b, h = c // 2, c % 2
        q0s = Q0S[h]
        xq = np.concatenate([iq[b, q0:q0 + 512] for q0 in q0s], axis=0)
        x = np.arange(P, dtype=np.float32)
        thr = np.empty((P, 2, 16), np.float32)
        for s_, q0 in enumerate(q0s):
            for kt in range(16):
                thr[:, s_, kt] = kt * P + x - q0
        in_maps.append({
            "wq": wq, "wk": wk, "wv": wv,
            "xqt": _sbufize(np.ascontiguousarray(xq.T)),
            "xkt": _sbufize(np.ascontiguousarray(ik[b].T)),
            "xvt": _sbufize(np.ascontiguousarray(iv[b].T)),
            "thr": thr, "iota": iota, "ones": onesm,
        })
    return in_maps


def _assemble(results):
    out = np.empty((B, S, D), np.float32)
    for c in range(NCORES):
        b, h = c // 2, c % 2
        oc = results[c]["out"].T        # [q_local, e]
        for s_, q0 in enumerate(Q0S[h]):
            out[b, q0:q0 + 512] = oc[s_ * 512:(s_ + 1) * 512]
    return out


def kernel(**inputs) -> np.ndarray:
    nc = _get_nc()
    in_maps = _host_prep(**inputs)
    res = run_bass_kernel_spmd(nc, in_maps, list(range(NCORES)))
    return _assemble(res.results)


def kernel_profiled(**inputs):
    """Like kernel() but also returns (output, exec_time_ns, results)."""
    nc = _get_nc()
    in_maps = _host_prep(**inputs)
    res = run_bass_kernel_spmd(nc, in_maps, list(range(NCORES)), trace=True)
    return _assemble(res.results), res.exec_time_ns, res


# revision 12
# speedup vs baseline: 2.4612x; 1.1231x over previous
"""Trainium2 Bass kernel for nn_AttentionHead (B=4, S=2048, D_IN=D_OUT=1024).

Sharding: 8 cores; core c handles batch b=c//2 and half the queries
(balanced for causal load): even cores q in [0,512)+[1536,2048), odd cores
q in [512,1536).  Each core computes full K^T and V projections for its
batch (duplicated within the pair) and causal attention for its queries,
as two uniform 512-query phase slots with K_slot=(8,16) key-tiles; causal
masking and the slot padding are data-driven (host-sent thresholds), so
all 8 cores run one identical SPMD program.

All matmuls are fp32r (full-rate TensorE at free-dim 512, ~2e-4 rel err).
Everything is computed transposed so no on-chip transposes are needed:
  K^T[e,k] = Wk-tiles.T @ Xk^T,  Q^T[e,q] = Wq-tiles.T @ Xq^T
  (host pre-transposes X into SBUF-ready [dp, do, s] blocks),
  V[k,e] = Xv^T-tiles.T @ Wv   (staged to DRAM, streamed back per e-tile),
  S^T[k,q] = KT-tiles.T @ QT,  exp+causal-mask on S^T,
  den[*,q] = ones.T @ expS (replicated on all partitions),
  O^T[e,q] = V-tiles.T @ expS^T, multiplied by 1/den.
"""
import sys
import types

sys.path.insert(0, "/opt/trn_rl_repo")


def _install_ntff_hook():
    import antenv

    if "antenv.axon_hooks" in sys.modules:
        return
    mod = types.ModuleType("antenv.axon_hooks")
    _h = [None]
    mod.set_axon_ntff_profile_hook = lambda h: _h.__setitem__(0, h)
    mod.get_axon_ntff_profile_hook = lambda: _h[0]
    sys.modules["antenv.axon_hooks"] = mod
    antenv.axon_hooks = mod
    try:
        from trn_agent_boot.trn_boot import _ntff_profile_via_ctypes

        mod.set_axon_ntff_profile_hook(
            _ntff_profile_via_ctypes("/opt/axon/libaxon_pjrt.so"))
    except Exception:
        pass


_install_ntff_hook()


import numpy as np
import concourse.bass as bass
import concourse.tile as tile
from concourse import mybir
from concourse.bass_utils import run_bass_kernel_spmd

P = 128
B, S, D = 4, 2048, 1024
N = 512                      # matmul moving free dim / queries per slot
NCORES = 8
K_SLOTS = (8, 16)            # k-tiles per phase slot (uniform across cores)
Q0S = {0: (0, 1536), 1: (512, 1024)}   # slot query starts per core parity
SCALE = float(1.0 / np.sqrt(np.float32(2048)))

f32 = mybir.dt.float32
f32r = mybir.dt.float32r
EXP = mybir.ActivationFunctionType.Exp
MULT = mybir.AluOpType.mult


def _split_multi_waits(nc):
    """Walrus allows one sync-wait per instruction; split extras onto
    wait-only NoOps inserted right before the offending instruction."""
    for f in nc.m.functions:
        for bb in f.blocks:
            insts = bb.instructions
            i = 0
            while i < len(insts):
                ins = insts[i]
                si = getattr(ins, "sync_info", None)
                if si and si.on_wait and len(si.on_wait) > 1:
                    waits = list(si.on_wait)
                    for j, w in enumerate(waits[:-1]):
                        nop = mybir.InstNoOp(
                            name=f"{ins.name}-waitsplit-{j}",
                            sync_info=mybir.SyncInfo(on_wait=[w], on_update=[]),
                            bass_nofuse=True,
                            engine=ins.engine, ins=[], outs=[])
                        insts.insert(i + j, nop)
                    i += len(waits) - 1
                    ins.sync_info = mybir.SyncInfo(
                        on_wait=[waits[-1]], on_update=list(si.on_update))
                i += 1


def build():
    nc = bass.Bass()
    # all host-side tensors are pre-arranged into SBUF layout [dp, do, cols]
    wq = nc.dram_tensor("wq", [P, 8, D], f32r, kind="ExternalInput")
    wk = nc.dram_tensor("wk", [P, 8, D], f32r, kind="ExternalInput")
    wv = nc.dram_tensor("wv", [P, 8, D], f32r, kind="ExternalInput")
    xqt = nc.dram_tensor("xqt", [P, 8, 1024], f32r, kind="ExternalInput")
    xkt = nc.dram_tensor("xkt", [P, 8, S], f32r, kind="ExternalInput")
    xvt = nc.dram_tensor("xvt", [P, 8, S], f32r, kind="ExternalInput")
    thr = nc.dram_tensor("thr", [P, 2, 16], f32, kind="ExternalInput")
    iot = nc.dram_tensor("iota", [P, N], f32, kind="ExternalInput")
    one_in = nc.dram_tensor("ones", [P, P], f32r, kind="ExternalInput")
    out = nc.dram_tensor("out", [D, 1024], f32, kind="ExternalOutput")

    with tile.TileContext(nc) as tc:
        from contextlib import ExitStack
        with ExitStack() as ctx:
            kt_pool = ctx.enter_context(tc.tile_pool(name="ktp", bufs=1))
            xh_pool = ctx.enter_context(tc.tile_pool(name="xh", bufs=2))
            sm_pool = ctx.enter_context(tc.tile_pool(name="sm", bufs=1))
            psum = ctx.enter_context(
                tc.tile_pool(name="ps", bufs=8, space="PSUM"))
            dram = ctx.enter_context(
                tc.tile_pool(name="dram", bufs=1, space="DRAM"))

            v_dram = dram.tile([P, 16, D], f32r)    # V: [k_p, k_o, e]

            KT = kt_pool.tile([P, 8, S], f32r)      # K^T: [e_p, e_o, k]

            ones = sm_pool.tile([P, P], f32r)
            nc.gpsimd.dma_start(ones[:], one_in[:])
            iota_sb = sm_pool.tile([P, N], f32)
            nc.gpsimd.dma_start(iota_sb[:], iot[:])
            thr_sb = sm_pool.tile([P, 2, 16], f32)
            nc.gpsimd.dma_start(thr_sb[:], thr[:])

            ET_GROUPS = ((0, 3), (3, 6), (6, 8))

            # ---- Stage A: K^T[e,k] = sum_d Wk-tiles.T @ Xk^T[d,k] ----
            # ---- Stage B: V[k,e]  = sum_d Xv^T-tiles.T @ Wv[d,e]  ----
            with tc.tile_pool(name="wres", bufs=1) as wres, \
                    tc.tile_pool(name="vp", bufs=3) as v_pool:
                wk_sb = wres.tile([P, 8, D], f32r, tag="w")
                for d in range(8):
                    nc.sync.dma_start(wk_sb[:, d, :], wk[:, d, :])
                for half in range(2):
                    xk_h = xh_pool.tile([P, 8, 1024], f32r, tag="xh",
                                        name=f"xk{half}")
                    for d in range(8):
                        nc.sync.dma_start(
                            xk_h[:, d, :],
                            xkt[:, d, half * 1024:(half + 1) * 1024])
                    # d-outer so the first matmul only needs strip d=0
                    for g0, g1 in ET_GROUPS:
                        pss = {}
                        for et in range(g0, g1):
                            for kc in range(2):
                                pss[(et, kc)] = psum.tile(
                                    [P, N], f32, tag="ps",
                                    name=f"psa{half}_{et}_{kc}")
                        for d in range(8):
                            for et in range(g0, g1):
                                lhs = wk_sb[:, d, et * P:(et + 1) * P]
                                for kc in range(2):
                                    nc.tensor.matmul(
                                        pss[(et, kc)][:], lhs,
                                        xk_h[:, d, kc * N:(kc + 1) * N],
                                        start=(d == 0), stop=(d == 7))
                        for et in range(g0, g1):
                            for kc in range(2):
                                col = half * 1024 + kc * N
                                if (et + kc) % 2 == 0:
                                    nc.vector.tensor_copy(
                                        KT[:, et, col:col + N],
                                        pss[(et, kc)][:])
                                else:
                                    nc.scalar.copy(
                                        KT[:, et, col:col + N],
                                        pss[(et, kc)][:])

                wv_sb = wres.tile([P, 8, D], f32r, tag="w")
                for d in range(8):
                    nc.sync.dma_start(wv_sb[:, d, :], wv[:, d, :])
                for half in range(2):
                    xv_h = xh_pool.tile([P, 8, 1024], f32r, tag="xh",
                                        name=f"xv{half}")
                    for d in range(8):
                        nc.sync.dma_start(
                            xv_h[:, d, :],
                            xvt[:, d, half * 1024:(half + 1) * 1024])
                    for ktl in range(8):
                        ktg = half * 8 + ktl
                        ps2 = [psum.tile([P, N], f32, tag="ps",
                                         name=f"psb{half}_{ktl}_{i}")
                               for i in range(2)]
                        for d in range(8):
                            lhs = xv_h[:, d, ktl * P:(ktl + 1) * P]
                            for ec in range(2):
                                nc.tensor.matmul(
                                    ps2[ec][:], lhs,
                                    wv_sb[:, d, ec * N:(ec + 1) * N],
                                    start=(d == 0), stop=(d == 7))
                        vt = v_pool.tile([P, D], f32r, tag="vst")
                        nc.vector.tensor_copy(vt[:, 0:N], ps2[0][:])
                        nc.scalar.copy(vt[:, N:D], ps2[1][:])
                        nc.gpsimd.dma_start(v_dram[:, ktg, :], vt[:])

            # ---- Stage D: per phase slot: Q^T, scores, softmax, O^T ----
            qt_pool = ctx.enter_context(tc.tile_pool(name="qtp", bufs=1))
            wq_pool = ctx.enter_context(tc.tile_pool(name="wqp", bufs=2))
            vin_pool = ctx.enter_context(tc.tile_pool(name="vin", bufs=3))
            out_pool = ctx.enter_context(tc.tile_pool(name="op", bufs=2))
            mk_pool = ctx.enter_context(tc.tile_pool(name="mk", bufs=2))
            rd_pool = ctx.enter_context(tc.tile_pool(name="rd", bufs=2))
            for s in range(2):
                K = K_SLOTS[s]
                # Q^T for this slot
                xq_s = xh_pool.tile([P, 8, N], f32r, tag="xh",
                                    name=f"xq{s}")
                nc.sync.dma_start(xq_s[:], xqt[:, :, s * N:(s + 1) * N])
                QT = qt_pool.tile([P, 8, N], f32r, tag="qt", name=f"qt{s}")
                for et in range(8):
                    wqt = wq_pool.tile([P, 8, P], f32r, tag="wqt",
                                       name=f"wqt{s}_{et}")
                    nc.gpsimd.dma_start(wqt[:], wq[:, :, et * P:(et + 1) * P])
                    psq = psum.tile([P, N], f32, tag="ps", name=f"psq{s}_{et}")
                    for d in range(8):
                        nc.tensor.matmul(psq[:], wqt[:, d, :], xq_s[:, d, :],
                                         start=(d == 0), stop=(d == 7))
                    if et % 2 == 0:
                        nc.vector.tensor_copy(QT[:, et, :], psq[:])
                    else:
                        nc.scalar.copy(QT[:, et, :], psq[:])

                # scores^T -> exp -> causal/pad mask
                expS = xh_pool.tile([P, 16, N], f32r, tag="xh",
                                    name=f"expS{s}")
                for kt in range(K):
                    ps = psum.tile([P, N], f32, tag="ps", name=f"pss{s}_{kt}")
                    for ec in range(8):
                        nc.tensor.matmul(
                            ps[:], KT[:, ec, kt * P:(kt + 1) * P],
                            QT[:, ec, :],
                            start=(ec == 0), stop=(ec == 7))
                    nc.scalar.activation(expS[:, kt, :], ps[:], EXP,
                                         scale=SCALE)
                    if not (s == 1 and kt < 8):
                        mk = mk_pool.tile([P, N], f32r)
                        nc.vector.tensor_scalar(
                            out=mk[:], in0=iota_sb[:],
                            scalar1=thr_sb[:, s, kt:kt + 1], scalar2=None,
                            op0=mybir.AluOpType.is_ge)
                        nc.vector.tensor_tensor(
                            out=expS[:, kt, :], in0=expS[:, kt, :],
                            in1=mk[:], op=MULT)

                # denominator, replicated on all partitions
                dps = psum.tile([P, N], f32, tag="ps", name=f"dps{s}")
                for kt in range(K):
                    nc.tensor.matmul(dps[:], ones[:], expS[:, kt, :],
                                     start=(kt == 0), stop=(kt == K - 1))
                rden = rd_pool.tile([P, N], f32)
                nc.vector.reciprocal(rden[:], dps[:])

                # O^T[e,q] with per-et V slabs streamed from DRAM
                for et in range(8):
                    slab = vin_pool.tile([P, 16, P], f32r, tag="vs",
                                         name=f"vs{s}_{et}")
                    nc.sync.dma_start(
                        slab[:, :K, :], v_dram[:, :K, et * P:(et + 1) * P])
                    po = psum.tile([P, N], f32, tag="ps", name=f"po{s}_{et}")
                    for kt in range(K):
                        nc.tensor.matmul(po[:], slab[:, kt, :],
                                         expS[:, kt, :],
                                         start=(kt == 0), stop=(kt == K - 1))
                    ot = out_pool.tile([P, N], f32)
                    nc.vector.tensor_tensor(out=ot[:], in0=po[:],
                                            in1=rden[:], op=MULT)
                    nc.gpsimd.dma_start(
                        out[et * P:(et + 1) * P, s * N:(s + 1) * N], ot[:])

    _split_multi_waits(nc)
    return nc


_NC_CACHE = None


def _get_nc():
    global _NC_CACHE
    if _NC_CACHE is None:
        _NC_CACHE = build()
    return _NC_CACHE


def _sbufize(a):
    """[rows(1024), cols] -> [dp(128), do(8), cols] contiguous."""
    r, c = a.shape
    return np.ascontiguousarray(a.reshape(8, P, c).transpose(1, 0, 2))


def _host_prep(inputs_for_keys, inputs_for_values, inputs_for_queries,
               weight_q, weight_k, weight_v):
    f = lambda a: np.asarray(a, dtype=np.float32)
    ik, iv, iq = f(inputs_for_keys), f(inputs_for_values), f(inputs_for_queries)
    wq = _sbufize(f(weight_q))
    wk = _sbufize(f(weight_k))
    wv = _sbufize(f(weight_v))

    iota = np.broadcast_to(np.arange(N, dtype=np.float32), (P, N)).copy()
    onesm = np.ones((P, P), np.float32)
    in_maps = []
    for c in range(NCORES):
        b, h = c // 2, c % 2
        q0s = Q0S[h]
        xq = np.concatenate([iq[b, q0:q0 + 512] for q0 in q0s], axis=0)
        x = np.arange(P, dtype=np.float32)
        thr = np.empty((P, 2, 16), np.float32)
        for s_, q0 in enumerate(q0s):
            for kt in range(16):
                thr[:, s_, kt] = kt * P + x - q0
        in_maps.append({
            "wq": wq, "wk": wk, "wv": wv,
            "xqt": _sbufize(np.ascontiguousarray(xq.T)),
            "xkt": _sbufize(np.ascontiguousarray(ik[b].T)),
            "xvt": _sbufize(np.ascontiguousarray(iv[b].T)),
            "thr": thr, "iota": iota, "ones": onesm,
        })
    return in_maps


def _assemble(results):
    out = np.empty((B, S, D), np.float32)
    for c in range(NCORES):
        b, h = c // 2, c % 2
        oc = results[c]["out"].T        # [q_local, e]
        for s_, q0 in enumerate(Q0S[h]):
            out[b, q0:q0 + 512] = oc[s_ * 512:(s_ + 1) * 512]
    return out


def kernel(**inputs) -> np.ndarray:
    nc = _get_nc()
    in_maps = _host_prep(**inputs)
    res = run_bass_kernel_spmd(nc, in_maps, list(range(NCORES)))
    return _assemble(res.results)


def kernel_profiled(**inputs):
    """Like kernel() but also returns (output, exec_time_ns, results)."""
    nc = _get_nc()
    in_maps = _host_prep(**inputs)
    res = run_bass_kernel_spmd(nc, in_maps, list(range(NCORES)), trace=True)
    return _assemble(res.results), res.exec_time_ns, res
